# revision 15
# baseline (speedup 1.0000x reference)
"""Trainium2 Bass kernel for AttentionPooling.

Math (per batch element b):
  xf = x[b] reshaped [C, N] with C=512, N=4096
  q = wq@xf + bq ; k = wk@xf + bk ; v = wv@xf + bv          (each [64, N])
  logits = q @ k^T  [64, 64];  attn = softmax(logits, axis over rows o)
  out[b] = mean_n(attn @ v) = attn @ mean_n(v)              ([64])

Because attn does not depend on n, mean_n(attn @ v) = attn @ vbar with
vbar = mean_n(v).  The kernel computes, per batch:
  - q0/k0 = (wq|wk) @ xf via f32r (tf32-like) matmuls, W-stationary, N=512
    tiles, accumulated over 4 C-chunks into PSUM (no bias).
  - PE-transposes of the [128, n] qk tiles into n-major layout.
  - A ones-augmented f32r matmul accumulating over all 32 n-subtiles:
      lhsT = [kT | 1], rhs = [1 | qT]  ->  [65, 65] PSUM holding
      L0^T = k0@q0^T, sk = sum_n k0, sq = sum_n q0.
  - Bias corrections applied analytically on the 64x64 logits:
      L^T = L0^T + bq (x) (sk + N bk) + bk (x) sq
  - v0 = wv@xf tiles reduced over n on the vector engine -> vbar.
  - Softmax along the free dim of L^T (scalar-engine exp with accumulate),
    folded denominator:  out = E^T @ (vbar / s).

Data-parallel over batch across the 8 NeuronCores (4 batch elements per
core); no collectives.
"""

import sys

import numpy as np

for _p in ("/opt/trn_rl_repo", "/root/.axon_site/_ro/trn_rl_repo"):
    if _p not in sys.path:
        sys.path.insert(0, _p)

import concourse.bacc as bacc
import concourse.mybir as mybir
import concourse.tile as tile
from concourse import masks
from concourse.bass_utils import run_bass_kernel_spmd

B, C, H, W = 32, 512, 64, 64
N = H * W            # 4096
C8 = 64              # C // 8
NCORES = 8
BPC = B // NCORES    # batch elements per core
NCHUNK = C // 128    # C chunks of 128
TW = 512             # projection tile width (PSUM bank = 512 f32)
NT = N // TW         # 8 projection tiles
NSUB = TW // 128     # transpose subtiles per projection tile

F32 = mybir.dt.float32
F32R = mybir.dt.float32r
AX = mybir.AxisListType.X
MULT = mybir.AluOpType.mult
ADD = mybir.AluOpType.add

_NC_CACHE = {}


def _build_nc():
    nc = bacc.Bacc("TRN2", target_bir_lowering=False, debug=False)

    x_d = nc.dram_tensor("x", [BPC, C, N], F32R, kind="ExternalInput")
    wq_d = nc.dram_tensor("wq", [C8, C], F32, kind="ExternalInput")
    bq_d = nc.dram_tensor("bq", [C8], F32, kind="ExternalInput")
    wk_d = nc.dram_tensor("wk", [C8, C], F32, kind="ExternalInput")
    bk_d = nc.dram_tensor("bk", [C8], F32, kind="ExternalInput")
    wv_d = nc.dram_tensor("wv", [C8, C], F32, kind="ExternalInput")
    bv_d = nc.dram_tensor("bv", [C8], F32, kind="ExternalInput")
    out_d = nc.dram_tensor("out", [BPC, C8], F32, kind="ExternalOutput")

    with tile.TileContext(nc, trace_sim=False) as tc:
        with (
            tc.tile_pool(name="const", bufs=1) as constp,
            tc.tile_pool(name="xpool", bufs=2) as xpool,
            tc.tile_pool(name="qkpool", bufs=3) as qkpool,
            tc.tile_pool(name="attpool", bufs=4) as attpool,
            tc.tile_pool(name="smallp", bufs=2) as smallp,
            tc.tile_pool(name="ps_qk", bufs=2, space="PSUM") as ps_qk,
            tc.tile_pool(name="ps_v", bufs=2, space="PSUM") as ps_v,
            tc.tile_pool(name="ps_t", bufs=2, space="PSUM") as ps_t,
            tc.tile_pool(name="ps_att", bufs=1, space="PSUM") as ps_att,
            tc.tile_pool(name="ps_small", bufs=1, space="PSUM") as ps_small,
        ):
            # ---------------- one-time prep ----------------
            ident = constp.tile([128, 128], F32)
            masks.make_identity(nc, ident[:])

            ones_row = constp.tile([1, C8], F32)
            nc.vector.memset(ones_row[:], 1.0)
            # static f32r ones pair for the attention-matmul border columns
            ones2_f32 = constp.tile([128, 2], F32)
            nc.vector.memset(ones2_f32[:], 1.0)
            ones2 = constp.tile([128, 2], F32R)
            nc.scalar.copy(ones2[:], ones2_f32[:])
            # ones at partition 64 (for broadcasting the sq row which the
            # big attention matmul leaves on PSUM partition 64)
            ones64 = constp.tile([C8 + 1, C8], F32)
            nc.vector.memset(ones64[C8 : C8 + 1, :], 1.0)

            wqk_raw = constp.tile([128, C], F32)  # [wq rows | wk rows]
            nc.sync.dma_start(wqk_raw[0:C8, :], wq_d.ap()[:, :])
            nc.sync.dma_start(wqk_raw[C8:128, :], wk_d.ap()[:, :])
            wv_raw = constp.tile([C8, C], F32)
            nc.sync.dma_start(wv_raw[:], wv_d.ap()[:, :])

            bq_row = constp.tile([1, C8], F32)
            nc.sync.dma_start(bq_row[:], bq_d.ap().unsqueeze(0))
            bk_row = constp.tile([1, C8], F32)
            nc.sync.dma_start(bk_row[:], bk_d.ap().unsqueeze(0))
            bv_row = constp.tile([1, C8], F32)
            nc.sync.dma_start(bv_row[:], bv_d.ap().unsqueeze(0))

            # transposed weight chunks: wqkT[c] = (wqk chunk)^T [128, 128]
            wqkT = []
            wvT = []
            for c in range(NCHUNK):
                pt = ps_small.tile([128, 128], F32, tag="sp")
                nc.tensor.transpose(
                    pt[:], wqk_raw[:, c * 128 : (c + 1) * 128], ident[:]
                )
                st = constp.tile([128, 128], F32R, tag=f"wqkT{c}")
                nc.scalar.copy(st[:], pt[:])
                wqkT.append(st)

                pv = ps_small.tile([128, C8], F32, tag="sp")
                nc.tensor.transpose(
                    pv[:], wv_raw[:, c * 128 : (c + 1) * 128], ident[0:C8, 0:C8]
                )
                sv = constp.tile([128, C8], F32R, tag=f"wvT{c}")
                nc.scalar.copy(sv[:], pv[:])
                wvT.append(sv)

            # bias-derived constants
            p_bc = ps_small.tile([C8, C8], F32, tag="sp")
            nc.tensor.matmul(p_bc[:], ones_row[:], bq_row[:], start=True, stop=True)
            bq_bc = constp.tile([C8, C8], F32)  # every row = bq
            nc.scalar.copy(bq_bc[:], p_bc[:])

            p_bk = ps_small.tile([C8, 1], F32, tag="sp")
            nc.tensor.matmul(
                p_bk[:], bk_row[:], ones_row[:, 0:1], start=True, stop=True
            )
            bk_col = constp.tile([C8, 1], F32)
            nc.scalar.copy(bk_col[:], p_bk[:])

            p_bv = ps_small.tile([C8, 1], F32, tag="sp")
            nc.tensor.matmul(
                p_bv[:], bv_row[:], ones_row[:, 0:1], start=True, stop=True
            )
            bv_col = constp.tile([C8, 1], F32)
            nc.scalar.copy(bv_col[:], p_bv[:])

            # ---------------- per batch element ----------------
            for b in range(BPC):
                xc = []
                for c in range(NCHUNK):
                    t = xpool.tile([128, N], F32R, tag=f"x{c}")
                    nc.sync.dma_start(t[:], x_d.ap()[b, c * 128 : (c + 1) * 128, :])
                    xc.append(t)

                # [65, 66]: f32r needs an even moving width; col 65 is junk
                att_ps = ps_att.tile([C8 + 1, C8 + 2], F32)
                vs_part = smallp.tile([C8, NT], F32, tag="vs_part")

                for ti in range(NT):
                    sl = slice(ti * TW, (ti + 1) * TW)
                    qk_ps = ps_qk.tile([128, TW], F32, tag="qk_ps")
                    for c in range(NCHUNK):
                        nc.tensor.matmul(
                            qk_ps[:],
                            wqkT[c][:],
                            xc[c][:, sl],
                            start=(c == 0),
                            stop=(c == NCHUNK - 1),
                        )
                    v_ps = ps_v.tile([C8, TW], F32, tag="v_ps")
                    for c in range(NCHUNK):
                        nc.tensor.matmul(
                            v_ps[:],
                            wvT[c][:],
                            xc[c][:, sl],
                            start=(c == 0),
                            stop=(c == NCHUNK - 1),
                        )
                    qk_sb = qkpool.tile([128, TW], F32, tag="qk_sb")
                    nc.scalar.copy(qk_sb[:], qk_ps[:])
                    # spatial sum of v0 for this tile (vector engine)
                    nc.vector.reduce_sum(
                        vs_part[:, ti : ti + 1], v_ps[:], axis=AX
                    )

                    for s in range(NSUB):
                        t_ps = ps_t.tile([128, 128], F32, tag="t_ps")
                        nc.tensor.transpose(
                            t_ps[:], qk_sb[:, s * 128 : (s + 1) * 128], ident[:]
                        )
                        # a_sb layout: [ones | qT | kT | ones], 130 cols
                        a_sb = attpool.tile([128, 130], F32R, tag="a_sb")
                        nc.vector.tensor_copy(a_sb[:, 0:130:129], ones2[:])
                        nc.vector.tensor_copy(a_sb[:, 1:129], t_ps[:])
                        first = ti == 0 and s == 0
                        last = ti == NT - 1 and s == NSUB - 1
                        # out[65,65]: [0:64,0]=sk, [0:64,1:65]=L0T,
                        #             [64,1:65]=sq, [64,0]=N
                        nc.tensor.matmul(
                            att_ps[:],
                            a_sb[:, 65:130],
                            a_sb[:, 0:66],
                            start=first,
                            stop=last,
                        )

                # ---------------- finalize batch b ----------------
                # skp = sk + N*bk
                skp = smallp.tile([C8, 1], F32, tag="skp")
                nc.vector.scalar_tensor_tensor(
                    skp[:], bk_col[:], float(N), att_ps[0:C8, 0:1], op0=MULT, op1=ADD
                )
                # broadcast sq (psum row at partition 64) to all partitions
                sq_sb = smallp.tile([C8 + 1, C8], F32, tag="sq_sb")
                nc.scalar.copy(sq_sb[C8 : C8 + 1, :], att_ps[C8 : C8 + 1, 1 : C8 + 1])
                sq_ps = ps_small.tile([C8, C8], F32, tag="sp")
                nc.tensor.matmul(
                    sq_ps[:],
                    ones64[C8 : C8 + 1, :],
                    sq_sb[C8 : C8 + 1, :],
                    start=True,
                    stop=True,
                )
                # LT = L0T + bq_bc * skp + sq_bc * bk
                L1 = smallp.tile([C8, C8], F32, tag="L1")
                nc.vector.scalar_tensor_tensor(
                    L1[:], bq_bc[:], skp[:], att_ps[0:C8, 1 : C8 + 1],
                    op0=MULT, op1=ADD,
                )
                LT = smallp.tile([C8, C8], F32, tag="LT")
                nc.vector.scalar_tensor_tensor(
                    LT[:], sq_ps[:], bk_col[:], L1[:], op0=MULT, op1=ADD
                )
                # softmax along free dim (the o axis)
                negm = smallp.tile([C8, 1], F32, tag="negm")
                nc.vector.reduce_max(negm[:], LT[:], axis=AX, negate=True)
                E = smallp.tile([C8, C8], F32, tag="E")
                s_col = smallp.tile([C8, 1], F32, tag="s_col")
                nc.scalar.activation(
                    E[:],
                    LT[:],
                    mybir.ActivationFunctionType.Exp,
                    bias=negm[:],
                    scale=1.0,
                    accum_out=s_col[:],
                )
                # vbar = vsum/N + bv
                vsum = smallp.tile([C8, 1], F32, tag="vsum")
                nc.vector.reduce_sum(vsum[:], vs_part[:], axis=AX)
                vbar = smallp.tile([C8, 1], F32, tag="vbar")
                nc.vector.scalar_tensor_tensor(
                    vbar[:], vsum[:], 1.0 / N, bv_col[:], op0=MULT, op1=ADD
                )
                # w = vbar / s ; out = E^T @ w  (as row via lhsT=w)
                rs = smallp.tile([C8, 1], F32, tag="rs")
                nc.vector.reciprocal(rs[:], s_col[:])
                wcol = smallp.tile([C8, 1], F32, tag="wcol")
                nc.vector.tensor_tensor(wcol[:], vbar[:], rs[:], op=MULT)
                out_ps = ps_small.tile([1, C8], F32, tag="sp")
                nc.tensor.matmul(out_ps[:], wcol[:], E[:], start=True, stop=True)
                out_row = smallp.tile([1, C8], F32, tag="out_row")
                nc.scalar.copy(out_row[:], out_ps[:])
                nc.sync.dma_start(out_d.ap()[b : b + 1, :], out_row[:])

    nc.compile()
    return nc


def _get_nc():
    if "nc" not in _NC_CACHE:
        _NC_CACHE["nc"] = _build_nc()
    return _NC_CACHE["nc"]


def _round_tf32(a):
    """Round-to-nearest onto the tf32 grid (10 explicit mantissa bits) so the
    PE's fp32r truncation is exact round-to-nearest."""
    a = np.ascontiguousarray(a, np.float32)
    i = a.view(np.uint32).astype(np.uint64)
    r = ((i + 0x1000 + ((i >> 13) & 1)) & 0xFFFFE000).astype(np.uint32)
    return r.view(np.float32)


def _make_in_maps(x, wq, bq, wk, bk, wv, bv):
    xf = _round_tf32(np.asarray(x, dtype=np.float32).reshape(B, C, N))
    shared = {
        "wq": _round_tf32(np.asarray(wq, np.float32)),
        "bq": np.asarray(bq, np.float32),
        "wk": _round_tf32(np.asarray(wk, np.float32)),
        "bk": np.asarray(bk, np.float32),
        "wv": _round_tf32(np.asarray(wv, np.float32)),
        "bv": np.asarray(bv, np.float32),
    }
    return [
        {"x": xf[i * BPC : (i + 1) * BPC], **shared} for i in range(NCORES)
    ]


def kernel(x, wq, bq, wk, bk, wv, bv):
    nc = _get_nc()
    in_maps = _make_in_maps(x, wq, bq, wk, bk, wv, bv)
    res = run_bass_kernel_spmd(nc, in_maps, core_ids=list(range(NCORES)))
    out = np.concatenate([res.results[i]["out"] for i in range(NCORES)], axis=0)
    return out.astype(np.float32)


# revision 19
# speedup vs baseline: 9.5834x; 9.5834x over previous
"""Trainium2 Bass kernel for AttentionPooling.

Math (per batch element b):
  xf = x[b] reshaped [C, N] with C=512, N=4096
  q = wq@xf + bq ; k = wk@xf + bk ; v = wv@xf + bv          (each [64, N])
  logits = q @ k^T  [64, 64];  attn = softmax(logits, axis over rows o)
  out[b] = mean_n(attn @ v) = attn @ mean_n(v)              ([64])

Because attn does not depend on n, mean_n(attn @ v) = attn @ vbar with
vbar = mean_n(v).  The kernel computes, per batch:
  - q0/k0 = (wq|wk) @ xf via f32r (tf32-like) matmuls, W-stationary, N=512
    tiles, accumulated over 4 C-chunks into PSUM (no bias).
  - PE-transposes of the [128, n] qk tiles into n-major layout.
  - A ones-augmented f32r matmul accumulating over all 32 n-subtiles:
      lhsT = [kT | 1], rhs = [1 | qT]  ->  [65, 65] PSUM holding
      L0^T = k0@q0^T, sk = sum_n k0, sq = sum_n q0.
  - Bias corrections applied analytically on the 64x64 logits:
      L^T = L0^T + bq (x) (sk + N bk) + bk (x) sq
  - v0 = wv@xf tiles reduced over n on the vector engine -> vbar.
  - Softmax along the free dim of L^T (scalar-engine exp with accumulate),
    folded denominator:  out = E^T @ (vbar / s).

Data-parallel over batch across the 8 NeuronCores (4 batch elements per
core); no collectives.
"""

import sys

import numpy as np

for _p in ("/opt/trn_rl_repo", "/root/.axon_site/_ro/trn_rl_repo"):
    if _p not in sys.path:
        sys.path.insert(0, _p)

import concourse.bacc as bacc
import concourse.mybir as mybir
import concourse.tile as tile
from concourse import masks
from concourse.bass_utils import run_bass_kernel_spmd

B, C, H, W = 32, 512, 64, 64
N = H * W            # 4096
C8 = 64              # C // 8
NCORES = 8
BPC = B // NCORES    # batch elements per core
NCHUNK = C // 128    # C chunks of 128
TW = 512             # projection tile width (PSUM bank = 512 f32)
NT = N // TW         # 8 projection tiles
NSUB = TW // 128     # transpose subtiles per projection tile

F32 = mybir.dt.float32
F32R = mybir.dt.float32r
AX = mybir.AxisListType.X
MULT = mybir.AluOpType.mult
ADD = mybir.AluOpType.add

_NC_CACHE = {}


def _build_nc(loop_n=None):
    """Build the bass program.  loop_n wraps the per-batch section in a
    device-side For_i loop (used only for timing: the NEFF then executes the
    whole workload loop_n times back-to-back, making device time measurable
    over the host dispatch overhead)."""
    nc = bacc.Bacc("TRN2", target_bir_lowering=False, debug=False)

    x_d = nc.dram_tensor("x", [BPC, C, N], F32R, kind="ExternalInput")
    wq_d = nc.dram_tensor("wq", [C8, C], F32, kind="ExternalInput")
    bq_d = nc.dram_tensor("bq", [C8], F32, kind="ExternalInput")
    wk_d = nc.dram_tensor("wk", [C8, C], F32, kind="ExternalInput")
    bk_d = nc.dram_tensor("bk", [C8], F32, kind="ExternalInput")
    wv_d = nc.dram_tensor("wv", [C8, C], F32, kind="ExternalInput")
    bv_d = nc.dram_tensor("bv", [C8], F32, kind="ExternalInput")
    out_d = nc.dram_tensor("out", [BPC, C8], F32, kind="ExternalOutput")

    with tile.TileContext(nc, trace_sim=False) as tc:
        with (
            tc.tile_pool(name="const", bufs=1) as constp,
            tc.tile_pool(name="xpool", bufs=2) as xpool,
            tc.tile_pool(name="qkpool", bufs=3) as qkpool,
            tc.tile_pool(name="attpool", bufs=4) as attpool,
            tc.tile_pool(name="smallp", bufs=2) as smallp,
            tc.tile_pool(name="ps_qk", bufs=2, space="PSUM") as ps_qk,
            tc.tile_pool(name="ps_v", bufs=2, space="PSUM") as ps_v,
            tc.tile_pool(name="ps_t", bufs=2, space="PSUM") as ps_t,
            tc.tile_pool(name="ps_att", bufs=1, space="PSUM") as ps_att,
            tc.tile_pool(name="ps_small", bufs=1, space="PSUM") as ps_small,
        ):
            # ---------------- one-time prep ----------------
            ident = constp.tile([128, 128], F32)
            masks.make_identity(nc, ident[:])

            ones_row = constp.tile([1, C8], F32)
            nc.vector.memset(ones_row[:], 1.0)
            # static f32r ones pair for the attention-matmul border columns
            ones2_f32 = constp.tile([128, 2], F32)
            nc.vector.memset(ones2_f32[:], 1.0)
            ones2 = constp.tile([128, 2], F32R)
            nc.scalar.copy(ones2[:], ones2_f32[:])
            # ones at partition 64 (for broadcasting the sq row which the
            # big attention matmul leaves on PSUM partition 64)
            ones64 = constp.tile([C8 + 1, C8], F32)
            nc.vector.memset(ones64[C8 : C8 + 1, :], 1.0)

            wqk_raw = constp.tile([128, C], F32)  # [wq rows | wk rows]
            nc.sync.dma_start(wqk_raw[0:C8, :], wq_d.ap()[:, :])
            nc.sync.dma_start(wqk_raw[C8:128, :], wk_d.ap()[:, :])
            wv_raw = constp.tile([C8, C], F32)
            nc.sync.dma_start(wv_raw[:], wv_d.ap()[:, :])

            bq_row = constp.tile([1, C8], F32)
            nc.sync.dma_start(bq_row[:], bq_d.ap().unsqueeze(0))
            bk_row = constp.tile([1, C8], F32)
            nc.sync.dma_start(bk_row[:], bk_d.ap().unsqueeze(0))
            bv_row = constp.tile([1, C8], F32)
            nc.sync.dma_start(bv_row[:], bv_d.ap().unsqueeze(0))

            # transposed weight chunks: wqkT[c] = (wqk chunk)^T [128, 128]
            wqkT = []
            wvT = []
            for c in range(NCHUNK):
                pt = ps_small.tile([128, 128], F32, tag="sp")
                nc.tensor.transpose(
                    pt[:], wqk_raw[:, c * 128 : (c + 1) * 128], ident[:]
                )
                st = constp.tile([128, 128], F32R, tag=f"wqkT{c}")
                nc.scalar.copy(st[:], pt[:])
                wqkT.append(st)

                pv = ps_small.tile([128, C8], F32, tag="sp")
                nc.tensor.transpose(
                    pv[:], wv_raw[:, c * 128 : (c + 1) * 128], ident[0:C8, 0:C8]
                )
                sv = constp.tile([128, C8], F32R, tag=f"wvT{c}")
                nc.scalar.copy(sv[:], pv[:])
                wvT.append(sv)

            # bias-derived constants
            p_bc = ps_small.tile([C8, C8], F32, tag="sp")
            nc.tensor.matmul(p_bc[:], ones_row[:], bq_row[:], start=True, stop=True)
            bq_bc = constp.tile([C8, C8], F32)  # every row = bq
            nc.scalar.copy(bq_bc[:], p_bc[:])

            p_bk = ps_small.tile([C8, 1], F32, tag="sp")
            nc.tensor.matmul(
                p_bk[:], bk_row[:], ones_row[:, 0:1], start=True, stop=True
            )
            bk_col = constp.tile([C8, 1], F32)
            nc.scalar.copy(bk_col[:], p_bk[:])

            p_bv = ps_small.tile([C8, 1], F32, tag="sp")
            nc.tensor.matmul(
                p_bv[:], bv_row[:], ones_row[:, 0:1], start=True, stop=True
            )
            bv_col = constp.tile([C8, 1], F32)
            nc.scalar.copy(bv_col[:], p_bv[:])

            # ---------------- per batch element ----------------
            def emit_batches():
                for b in range(BPC):
                    emit_batch(b)

            def emit_batch(b):
                xc = []
                for c in range(NCHUNK):
                    t = xpool.tile([128, N], F32R, tag=f"x{c}")
                    nc.sync.dma_start(t[:], x_d.ap()[b, c * 128 : (c + 1) * 128, :])
                    xc.append(t)

                # [65, 66]: f32r needs an even moving width; col 65 is junk
                att_ps = ps_att.tile([C8 + 1, C8 + 2], F32)
                vs_part = smallp.tile([C8, NT], F32, tag="vs_part")

                for ti in range(NT):
                    sl = slice(ti * TW, (ti + 1) * TW)
                    qk_ps = ps_qk.tile([128, TW], F32, tag="qk_ps")
                    for c in range(NCHUNK):
                        nc.tensor.matmul(
                            qk_ps[:],
                            wqkT[c][:],
                            xc[c][:, sl],
                            start=(c == 0),
                            stop=(c == NCHUNK - 1),
                        )
                    v_ps = ps_v.tile([C8, TW], F32, tag="v_ps")
                    for c in range(NCHUNK):
                        nc.tensor.matmul(
                            v_ps[:],
                            wvT[c][:],
                            xc[c][:, sl],
                            start=(c == 0),
                            stop=(c == NCHUNK - 1),
                        )
                    qk_sb = qkpool.tile([128, TW], F32, tag="qk_sb")
                    nc.scalar.copy(qk_sb[:], qk_ps[:])
                    # spatial sum of v0 for this tile (vector engine)
                    nc.vector.reduce_sum(
                        vs_part[:, ti : ti + 1], v_ps[:], axis=AX
                    )

                    for s in range(NSUB):
                        t_ps = ps_t.tile([128, 128], F32, tag="t_ps")
                        nc.tensor.transpose(
                            t_ps[:], qk_sb[:, s * 128 : (s + 1) * 128], ident[:]
                        )
                        # a_sb layout: [ones | qT | kT | ones], 130 cols
                        a_sb = attpool.tile([128, 130], F32R, tag="a_sb")
                        nc.vector.tensor_copy(a_sb[:, 0:130:129], ones2[:])
                        nc.vector.tensor_copy(a_sb[:, 1:129], t_ps[:])
                        first = ti == 0 and s == 0
                        last = ti == NT - 1 and s == NSUB - 1
                        # out[65,65]: [0:64,0]=sk, [0:64,1:65]=L0T,
                        #             [64,1:65]=sq, [64,0]=N
                        nc.tensor.matmul(
                            att_ps[:],
                            a_sb[:, 65:130],
                            a_sb[:, 0:66],
                            start=first,
                            stop=last,
                        )

                # ---------------- finalize batch b ----------------
                # skp = sk + N*bk
                skp = smallp.tile([C8, 1], F32, tag="skp")
                nc.vector.scalar_tensor_tensor(
                    skp[:], bk_col[:], float(N), att_ps[0:C8, 0:1], op0=MULT, op1=ADD
                )
                # broadcast sq (psum row at partition 64) to all partitions
                sq_sb = smallp.tile([C8 + 1, C8], F32, tag="sq_sb")
                nc.scalar.copy(sq_sb[C8 : C8 + 1, :], att_ps[C8 : C8 + 1, 1 : C8 + 1])
                sq_ps = ps_small.tile([C8, C8], F32, tag="sp")
                nc.tensor.matmul(
                    sq_ps[:],
                    ones64[C8 : C8 + 1, :],
                    sq_sb[C8 : C8 + 1, :],
                    start=True,
                    stop=True,
                )
                # LT = L0T + bq_bc * skp + sq_bc * bk
                L1 = smallp.tile([C8, C8], F32, tag="L1")
                nc.vector.scalar_tensor_tensor(
                    L1[:], bq_bc[:], skp[:], att_ps[0:C8, 1 : C8 + 1],
                    op0=MULT, op1=ADD,
                )
                LT = smallp.tile([C8, C8], F32, tag="LT")
                nc.vector.scalar_tensor_tensor(
                    LT[:], sq_ps[:], bk_col[:], L1[:], op0=MULT, op1=ADD
                )
                # softmax along free dim (the o axis)
                negm = smallp.tile([C8, 1], F32, tag="negm")
                nc.vector.reduce_max(negm[:], LT[:], axis=AX, negate=True)
                E = smallp.tile([C8, C8], F32, tag="E")
                s_col = smallp.tile([C8, 1], F32, tag="s_col")
                nc.scalar.activation(
                    E[:],
                    LT[:],
                    mybir.ActivationFunctionType.Exp,
                    bias=negm[:],
                    scale=1.0,
                    accum_out=s_col[:],
                )
                # vbar = vsum/N + bv
                vsum = smallp.tile([C8, 1], F32, tag="vsum")
                nc.vector.reduce_sum(vsum[:], vs_part[:], axis=AX)
                vbar = smallp.tile([C8, 1], F32, tag="vbar")
                nc.vector.scalar_tensor_tensor(
                    vbar[:], vsum[:], 1.0 / N, bv_col[:], op0=MULT, op1=ADD
                )
                # w = vbar / s ; out = E^T @ w  (as row via lhsT=w)
                rs = smallp.tile([C8, 1], F32, tag="rs")
                nc.vector.reciprocal(rs[:], s_col[:])
                wcol = smallp.tile([C8, 1], F32, tag="wcol")
                nc.vector.tensor_tensor(wcol[:], vbar[:], rs[:], op=MULT)
                out_ps = ps_small.tile([1, C8], F32, tag="sp")
                nc.tensor.matmul(out_ps[:], wcol[:], E[:], start=True, stop=True)
                out_row = smallp.tile([1, C8], F32, tag="out_row")
                nc.scalar.copy(out_row[:], out_ps[:])
                nc.sync.dma_start(out_d.ap()[b : b + 1, :], out_row[:])

            if loop_n is None:
                emit_batches()
            else:
                hints = (
                    mybir.EngineType.PE,
                    mybir.EngineType.DVE,
                    mybir.EngineType.Activation,
                    mybir.EngineType.SP,
                    mybir.EngineType.Pool,
                )
                with tc.For_i(0, loop_n, 1, hint_engines=hints):
                    emit_batches()

    nc.compile()
    return nc


def _get_nc(loop_n=None):
    key = ("nc", loop_n)
    if key not in _NC_CACHE:
        _NC_CACHE[key] = _build_nc(loop_n)
    return _NC_CACHE[key]


def _round_tf32(a):
    """Round-to-nearest onto the tf32 grid (10 explicit mantissa bits) so the
    PE's fp32r truncation is exact round-to-nearest."""
    a = np.ascontiguousarray(a, np.float32)
    i = a.view(np.uint32).astype(np.uint64)
    r = ((i + 0x1000 + ((i >> 13) & 1)) & 0xFFFFE000).astype(np.uint32)
    return r.view(np.float32)


def _make_in_maps(x, wq, bq, wk, bk, wv, bv):
    xf = _round_tf32(np.asarray(x, dtype=np.float32).reshape(B, C, N))
    shared = {
        "wq": _round_tf32(np.asarray(wq, np.float32)),
        "bq": np.asarray(bq, np.float32),
        "wk": _round_tf32(np.asarray(wk, np.float32)),
        "bk": np.asarray(bk, np.float32),
        "wv": _round_tf32(np.asarray(wv, np.float32)),
        "bv": np.asarray(bv, np.float32),
    }
    return [
        {"x": xf[i * BPC : (i + 1) * BPC], **shared} for i in range(NCORES)
    ]


def kernel(x, wq, bq, wk, bk, wv, bv):
    nc = _get_nc()
    in_maps = _make_in_maps(x, wq, bq, wk, bk, wv, bv)
    res = run_bass_kernel_spmd(nc, in_maps, core_ids=list(range(NCORES)))
    out = np.concatenate([res.results[i]["out"] for i in range(NCORES)], axis=0)
    return out.astype(np.float32)
